# revision 32
# baseline (speedup 1.0000x reference)
"""Trainium2 Bass kernel for nn_MHAttentionLayer_64587718197528.

Reference computation (B=4, L=1024, D_MODEL=1024, S=2048, T=NUM_TOKENS=1000,
H=16, E=256, D_LLM=4096):
    q = (X @ Wq.T + bq)            [B*L, H*E]      X = target_embedding
    k = (SE @ Wk.T + bk)           [S, H*E]        SE = source_embedding
    v = (VE @ Wv.T + bv)           [S, H*E]        VE = value_embedding
    scores[b,h,l,s] = q . k / 16 ; A = softmax_s ; out = A @ v
    y = out @ Wo.T + bo            [B*L, D_LLM]

Sharding: tensor-parallel over heads. Core i owns heads {2i, 2i+1} (an
e-slice of 512 of the H*E dim). Each core computes its q/k/v projections,
attention for its 2 heads, and a partial out-projection
  partial_i = attn_out_i @ Wo[:, sl_i].T          [B*L, D_LLM]
The host sums the 8 partials and adds bo (linearity of the projection).

Precision strategy (gate is rel 2e-2; measured 1.85e-2):
  fp8e4 + DoubleRow (2x PE rate, HW-verified 2.03x) for the q-projection,
  k-projection and the scores matmul -- their quantization error is
  attenuated by the softmax. The v-projection, AV and out-projection
  matmuls stay bf16 (fp8 there costs 2-4e-2 alone). fp8 operands carry
  scales chosen so maxima sit at ~150-170 of the e4m3 max 240 (TRN e4m3
  overflows to Inf at >240); descales fold into the eviction ACTs and
  the exp scale.

All inputs are pre-rearranged HOST-SIDE into the exact SBUF tile layouts
(chunk-major, partition rows of 4-8KB) so every DMA is a plain 2D copy
with maximal packet sizes -- the natural gather layouts ran the DMA
engines at ~50% utilization and made the KV phase DMA-bound. Streams are
spread over three queues: st/wq/xq0 on the Scalar engine's queue, vt on
the GpSimd engine's, everything else on Sync.

Phases:
  KV:   all four kproj chain groups first (fp8 DR, they need only the
        2MB st stream), then the v-chains (bf16) at the rate the vt
        stream arrives. kT lives per head as [128, 2*S] fp8 (free layout
        (et, s)); v as 4 tiles [128, 4*EC] bf16. Wo is NOT prefetched
        here (it competed with st/vt); it loads during attn.
  Attn: flat pipeline over the 16 (l-chunk, head) units. Per head:
        8 score steps, each 2 fp8-DR matmuls (kt stationary [128,2,128],
        qt moving [128,2,512]) into a double-buffered PSUM pair, exp on
        ACT (scale folds the fp8 descales; no max subtraction), AV bf16
        software-pipelined AV_DELAY steps behind, denominators via DVE
        accumulation + ones-matmul partition-reduce-broadcast +
        reciprocal. The exp chain lags the DR scores during each head's
        ramp (first ~3 steps), so the PREVIOUS head's AV tail (3 av
        pairs), its softmax finalize, and the next l-chunk's qproj
        chains are emitted inside the ramp as filler.
  Proj: partial = outT.T @ WoT per [128,512] tile (bf16), lt-outer so
        each 128-row band leaves as one 1MB DMA, PSUM evictions
        alternating between Scalar and Vector engines.
"""
import numpy as np

# ---- problem constants (hardcoded per contract) ----
B, L, D = 4, 1024, 1024
S, T = 2048, 1000
H, E = 16, 256
DL = 4096
BL = B * L            # 4096 query rows
EC = 512              # e-slice per core (2 heads)
NCORES = 8
TP = 1024             # T zero-padded (includes the v-bias ones row at 1000)
NKD = 8               # k-tiles for D=1024
NKT = 8               # k-tiles for TP=1024
NLC = BL // 512       # 8 l-chunks
NSC = S // 512        # 4 s-chunks (KV phase)

_CACHE = {}
MM_DTYPE = "bf16"     # dtype of the non-fp8 matmul pipeline
AV_DELAY = 3          # AV matmuls trail the scores by this many steps
DEBUG_DUMP = False    # extra DRAM outputs (qt/kt/o) for numerics debugging

# fp8 scales (host-side pre-scaling; descales fold into ACT scale args)
S_X = 32.0            # target_embedding (|X| < 5.3 -> < 170)
S_WQ = 4096.0         # Wq (|Wq| <= 1/32 -> <= 128)
S_SE = 32.0           # source_embedding
S_WK = 4096.0         # Wk (|Wk| <= 1/sqrt(1000) -> <= 130)
S_QO = 48.0           # q in fp8 (|q| < 3.3 -> < 160)
S_KO = 48.0           # k in fp8
SC_EVQ = S_QO / (S_X * S_WQ)
SC_EVK = S_KO / (S_SE * S_WK)
EXP_SCALE = 0.0625 / (S_QO * S_KO)


def _build_nc():
    from contextlib import ExitStack

    import concourse.tile as tile
    from concourse import bacc, bass_isa, mybir

    F32 = mybir.dt.float32
    F32R = mybir.dt.float32r
    MMD = mybir.dt.bfloat16 if MM_DTYPE == "bf16" else F32R
    FP8 = mybir.dt.float8e4
    AF = mybir.ActivationFunctionType
    MUL = mybir.AluOpType.mult
    ADD = mybir.AluOpType.add
    DR = mybir.MatmulPerfMode.DoubleRow

    nc = bacc.Bacc("TRN2", target_bir_lowering=False, debug=False,
                   num_devices=NCORES)

    # all inputs pre-rearranged host-side to SBUF layouts (see _prep)
    xt = nc.dram_tensor("xt", [NLC * 128, NKD * 512], FP8,
                        kind="ExternalInput")
    st = nc.dram_tensor("st", [NSC * 128, NKT * 512], FP8,
                        kind="ExternalInput")
    vt = nc.dram_tensor("vt", [NSC * 128, NKT * 512], MMD,
                        kind="ExternalInput")
    wqt = nc.dram_tensor("wqt", [128, NKD * EC], FP8, kind="ExternalInput")
    wkt = nc.dram_tensor("wkt", [128, NKT * EC], FP8, kind="ExternalInput")
    wvt = nc.dram_tensor("wvt", [128, NKT * EC], MMD, kind="ExternalInput")
    wot = nc.dram_tensor("wot", [EC, DL], MMD, kind="ExternalInput")
    bqbk_d = nc.dram_tensor("bqbk", [128, 8], F32, kind="ExternalInput")
    out_d = nc.dram_tensor("out", [BL, DL], MMD, kind="ExternalOutput")
    if DEBUG_DUMP:
        kt_dump = nc.dram_tensor("kt_dump", [256, 2 * S], FP8,
                                 kind="ExternalOutput")
        qt_dump = nc.dram_tensor("qt_dump", [256, 1024], FP8,
                                 kind="ExternalOutput")
        o_dump = nc.dram_tensor("o_dump", [EC, BL], MMD,
                                kind="ExternalOutput")

    NLT = BL // 128    # 32 l-tiles
    NST = S // 128     # 16 s-tiles
    NDC = DL // 512    # 8 out-proj chunks
    NSTP = NST // 2    # 8 score steps (2 s-tiles each)

    with tile.TileContext(nc) as tc:
        with ExitStack() as root:
            root.enter_context(
                nc.allow_low_precision(reason="bf16/fp8 matmul pipeline"))

            # ---- persistent pools ----
            consts = root.enter_context(tc.tile_pool(name="consts", bufs=1))
            kvp = root.enter_context(tc.tile_pool(name="kv", bufs=1))
            outp = root.enter_context(tc.tile_pool(name="outT", bufs=1))

            ones_f = consts.tile([128, 128], F32, name="ones_f")
            nc.vector.memset(ones_f[:], 1.0)
            ones_m = consts.tile([128, 128], MMD, name="ones_m")
            nc.vector.tensor_copy(ones_m[:], ones_f[:])

            bqbk_t = consts.tile([128, 8], F32, name="bqbk_t")
            bq_t = bqbk_t[:, 0:4]
            bk_t = bqbk_t[:, 4:8]

            # kT: per head [128, 2*S] fp8 (free layout (et, s));
            # v: 4 tiles [128, 4*EC] bf16 (4 s-tiles each)
            kt_sb = [kvp.tile([128, 2 * S], FP8, name=f"kt{h}", tag=f"kt{h}")
                     for h in range(2)]
            v_sb = [kvp.tile([128, 4 * EC], MMD, name=f"v{g}", tag=f"v{g}")
                    for g in range(4)]
            # outT: 4 e-tiles x [128, BL]
            o_sb = [outp.tile([128, BL], MMD, name=f"oT{m}", tag=f"oT{m}")
                    for m in range(4)]

            wq_pool = root.enter_context(tc.tile_pool(name="wq", bufs=1))
            wo_pool = root.enter_context(tc.tile_pool(name="wo", bufs=1))
            xq_pool = root.enter_context(tc.tile_pool(name="xq", bufs=2))
            pev_pool = root.enter_context(tc.tile_pool(name="pev", bufs=4))
            # qt + the misc PSUM pool live at root so the first qproj can
            # be emitted inside the KV phase (between k- and v-chains,
            # while the vt stream is still arriving)
            qt_pool = root.enter_context(tc.tile_pool(name="qtp", bufs=2))
            ps_misc_p = root.enter_context(
                tc.tile_pool(name="ps_misc", bufs=2, space="PSUM"))
            wq_t = []
            wo_sb = wo_pool.tile([128, 4 * DL], MMD, name="wo_sb")

            qt_tiles = {}

            def make_qt(lc):
                qt_tiles[lc] = [
                    qt_pool.tile([128, 2 * 512], FP8, tag=f"qt{h}",
                                 name=f"qt{h}_{lc}") for h in range(2)]

            # PE clock warm-up: the tensor engine ramps 0.65 -> 2.4GHz
            # over ~3-4us of sustained work; burn that in during the
            # initial DMA wait (~11us) on dummy matmuls nothing reads.
            # Uses the root misc PSUM pool (no extra bank).
            for wchain in range(6):
                wt = ps_misc_p.tile([128, 512], F32, tag="m", name="warm")
                for r in range(8):
                    nc.tensor.matmul(wt[:, 0:128], ones_m[:], ones_m[:],
                                     start=(r == 0), stop=(r == 7))

            def qproj_chain(lc, half, mh, qt_t):
                # one fp8 DoubleRow chain (one e-tile of the head)
                xq_t = xq_tiles[lc]
                wqv = wq_t[0][:].rearrange("p (k e) -> p k e", k=NKD)
                xqv = xq_t[:].rearrange("p (k c) -> p k c", k=NKD)
                m = half * 2 + mh
                ps_q = ps_misc_p.tile([128, 512], F32, tag="m",
                                      name=f"psq{mh}")
                for kj in range(NKD // 2):
                    nc.tensor.matmul(
                        ps_q[:],
                        wqv[:, 2 * kj:2 * kj + 2,
                            m * 128:(m + 1) * 128],
                        xqv[:, 2 * kj:2 * kj + 2, :],
                        start=(kj == 0), stop=(kj == NKD // 2 - 1),
                        perf_mode=DR)
                nc.scalar.activation(
                    qt_t[:, mh * 512:(mh + 1) * 512], ps_q[:],
                    AF.Identity, bias=bq_t[:, m:m + 1], scale=SC_EVQ)

            xq_tiles = {}

            def load_xq(lc, eng=None):
                t = xq_pool.tile([128, NKD * 512], FP8, tag="xq",
                                 name=f"xq{lc}")
                (eng or nc.sync).dma_start(
                    t[:], xt.ap()[lc * 128:(lc + 1) * 128, :])
                xq_tiles[lc] = t

            # ---- phase KV ----
            with ExitStack() as ph:
                ph.enter_context(nc.named_scope("kvproj"))
                wkv_pool = ph.enter_context(tc.tile_pool(name="wkv", bufs=1))
                sk_pool = ph.enter_context(tc.tile_pool(name="sk", bufs=4))
                sv_pool = ph.enter_context(tc.tile_pool(name="sv", bufs=4))
                psk = ph.enter_context(
                    tc.tile_pool(name="psk", bufs=1, space="PSUM"))
                psv = ph.enter_context(
                    tc.tile_pool(name="psv", bufs=1, space="PSUM"))
                wk_sb = wkv_pool.tile([128, NKT * EC], FP8, name="wk_sb")
                wv_sb = wkv_pool.tile([128, NKT * EC], MMD, name="wv_sb")

                def load_st(sc, eng=None):
                    t = sk_pool.tile([128, NKT * 512], FP8, tag="stg",
                                     name=f"stg{sc}")
                    (eng or nc.sync).dma_start(
                        t[:], st.ap()[sc * 128:(sc + 1) * 128, :])
                    return t

                def load_vt(sc, eng=None):
                    t = sv_pool.tile([128, NKT * 512], MMD, tag="vtg",
                                     name=f"vtg{sc}")
                    (eng or nc.sync).dma_start(
                        t[:], vt.ap()[sc * 128:(sc + 1) * 128, :])
                    return t

                # Per-core DMA bandwidth (~360GB/s) is shared across the
                # queues, so per-queue priority = issue order and the
                # three queues (Sync/Scalar/GpSimd) run in parallel.
                # Deadline order: wk+st (k-chains) < wq/xq0 (qproj0) <
                # wv+vt0 < vt1..3 (v-chains). Every DMA is a whole-chunk
                # 2D copy (4-8KB descriptors -- small descriptors lose
                # queue arbitration). k-chains run in st arrival order
                # 0,2,1,3 (two st streams in parallel).
                st_ts = {0: load_st(0, eng=nc.gpsimd),
                         2: load_st(2, eng=nc.scalar),
                         1: load_st(1, eng=nc.gpsimd),
                         3: load_st(3, eng=nc.scalar)}
                nc.sync.dma_start(wk_sb[:], wkt.ap())
                nc.sync.dma_start(bqbk_t[:], bqbk_d.ap())
                nc.sync.dma_start(wv_sb[:], wvt.ap())
                wq_sb = wq_pool.tile([128, NKD * EC], FP8, name="wq_sb")
                nc.scalar.dma_start(wq_sb[:], wqt.ap())
                wq_t.append(wq_sb)
                load_xq(0, eng=nc.scalar)
                vt_ts = {0: load_vt(0, eng=nc.gpsimd),
                         1: load_vt(1, eng=nc.sync),
                         2: load_vt(2, eng=nc.gpsimd),
                         3: load_vt(3, eng=nc.scalar)}
                # all k-chains first (m-major so evictions start early)
                wkv = wk_sb[:].rearrange("p (k e) -> p k e", k=NKT)
                for sc in (0, 2, 1, 3):
                    ps_k = [psk.tile([128, 512], F32, tag=f"psk{m}",
                                     name=f"psk{m}") for m in range(4)]
                    stv = st_ts[sc][:].rearrange("p (k c) -> p k c", k=NKT)
                    for m in range(4):
                        for kj in range(NKT // 2):
                            nc.tensor.matmul(
                                ps_k[m][:],
                                wkv[:, 2 * kj:2 * kj + 2,
                                    m * 128:(m + 1) * 128],
                                stv[:, 2 * kj:2 * kj + 2, :],
                                start=(kj == 0), stop=(kj == NKT // 2 - 1),
                                perf_mode=DR)
                        nc.scalar.activation(
                            kt_sb[m // 2][:, (m % 2) * S + sc * 512:
                                          (m % 2) * S + (sc + 1) * 512],
                            ps_k[m][:], AF.Identity, bias=bk_t[:, m:m + 1],
                            scale=SC_EVK)
                # l-chunk 0's q-projection here: PE work that needs only
                # wq/xq0, bridging the window where vt is still streaming
                make_qt(0)
                for half in range(2):
                    for mh in range(2):
                        qproj_chain(0, half, mh, qt_tiles[0][half])
                # v-chains at the rate the vt stream arrives (2 PSUM
                # tiles: j and j+2 share a bank, eviction-paced)
                for sc in range(NSC):
                    vt_t = vt_ts[sc]
                    for j in range(4):
                        pv = psv.tile([128, 512], F32, tag=f"psv{j % 2}",
                                      name=f"psv{j % 2}")
                        for kk in range(NKT):
                            nc.tensor.matmul(
                                pv[:],
                                vt_t[:, kk * 512 + j * 128:
                                     kk * 512 + (j + 1) * 128],
                                wv_sb[:, kk * EC:(kk + 1) * EC],
                                start=(kk == 0), stop=(kk == NKT - 1))
                        nc.scalar.activation(
                            v_sb[sc][:, j * EC:(j + 1) * EC], pv[:],
                            AF.Copy)
                if DEBUG_DUMP:
                    for h in range(2):
                        nc.sync.dma_start(
                            kt_dump[h * 128:(h + 1) * 128, :], kt_sb[h][:])

            # ---- fused attention phase ----
            with ExitStack() as ph:
                ph.enter_context(nc.named_scope("attn"))
                a_pool = ph.enter_context(tc.tile_pool(name="ap", bufs=1))
                acc_pool = ph.enter_context(tc.tile_pool(name="accp", bufs=2))
                bc_pool = ph.enter_context(tc.tile_pool(name="bcp", bufs=2))
                # PSUM budget (8 banks): misc (qproj + denom, root) 2,
                # scores double-buffered 4, attn-out accumulators 2.
                ps_sT_p = ph.enter_context(
                    tc.tile_pool(name="ps_sT", bufs=2, space="PSUM"))
                ps_o_p = ph.enter_context(
                    tc.tile_pool(name="ps_o", bufs=2, space="PSUM"))

                def attn_head_main(lc, h, qt_t, a_t, inserts=None):
                    # scoresT via one fp8 DoubleRow matmul per s-tile;
                    # one exp per 1024 columns; AV (bf16) pipelined
                    # AV_DELAY steps behind. The AV tail (last AV_DELAY
                    # steps) is NOT emitted here -- the caller threads it
                    # into the next head's ramp as PE filler.
                    acc = acc_pool.tile([128, 1024], F32, tag="acc",
                                        name="acc")
                    ps_os = [ps_o_p.tile([128, 512], F32, tag="ps_o",
                                         name="ps_o") for _ in range(2)]
                    ktv = kt_sb[h][:].rearrange("p (et s) -> p et s", et=2)
                    qtv = qt_t[:].rearrange("p (et l) -> p et l", et=2)

                    def av_pair(stp):
                        for et in range(2):
                            for sub in range(2):
                                stt = 2 * stp + sub
                                nc.tensor.matmul(
                                    ps_os[et][:],
                                    v_sb[stt // 4][:, (stt % 4) * EC + h * E
                                                   + et * 128:
                                                   (stt % 4) * EC + h * E
                                                   + (et + 1) * 128],
                                    a_t[stp][:, sub * 512:(sub + 1) * 512],
                                    start=(stt == 0), stop=(stt == NST - 1))

                    for stp in range(NSTP):
                        ps_sT = ps_sT_p.tile([128, 1024], F32, tag="ps_sT",
                                             name="ps_sT")
                        for sub in range(2):
                            stt = 2 * stp + sub
                            nc.tensor.matmul(
                                ps_sT[:, sub * 512:(sub + 1) * 512],
                                ktv[:, :, stt * 128:(stt + 1) * 128],
                                qtv,
                                start=True, stop=True, perf_mode=DR)
                        a_ap = a_t[stp][:]
                        nc.scalar.activation(a_ap, ps_sT[:], AF.Exp,
                                             scale=EXP_SCALE)
                        # accumulate denominator on DVE
                        if stp == 0:
                            nc.vector.tensor_copy(acc[:], a_ap)
                        else:
                            nc.vector.tensor_tensor(acc[:], acc[:], a_ap,
                                                    ADD)
                        if inserts and stp in inserts:
                            for fn in inserts[stp]:
                                fn()
                        if stp >= AV_DELAY:
                            av_pair(stp - AV_DELAY)
                    return acc, ps_os, av_pair

                def attn_fin(lc, h, acc, ps_os):
                    # softmax denominators: fold acc halves on DVE (f32),
                    # partition-reduce-with-broadcast on the (idle) GpSimd
                    # engine -- zero PE instructions -- then reciprocal on
                    # DVE.
                    accb = bc_pool.tile([128, 512], F32, tag="accb",
                                        name="accb")
                    nc.vector.tensor_tensor(accb[:], acc[:, 0:512],
                                            acc[:, 512:1024], ADD)
                    bcs = bc_pool.tile([128, 512], F32, tag="bcs",
                                       name="bcs")
                    nc.gpsimd.partition_all_reduce(
                        bcs[:], accb[:], 128, bass_isa.ReduceOp.add)
                    bc = bc_pool.tile([128, 512], F32, tag="bc", name="bc")
                    nc.vector.reciprocal_approx_fast(out=bc[:], in_=bcs[:])
                    for et in range(2):
                        m = 2 * h + et
                        nc.vector.tensor_tensor(
                            o_sb[m][:, lc * 512:(lc + 1) * 512],
                            ps_os[et][:], bc[:], MUL)

                # Flat pipeline over the 16 heads. Each head's ramp hosts
                # the PREVIOUS head's AV tail (steps 0..AV_DELAY-1), its
                # finalize (step AV_DELAY, always before this head's own
                # first av_pair), and the next l-chunk's qproj chains.
                # (qproj for l-chunk 0 already ran inside the KV phase.)
                if DEBUG_DUMP:
                    for h in range(2):
                        nc.sync.dma_start(
                            qt_dump[h * 128:(h + 1) * 128, :],
                            qt_tiles[0][h][:])

                # out-projection units for the first l-band, usable as PE
                # filler in the last l-chunk's ramps (no qproj there);
                # evictions on DVE (ACT is exp-critical in the ramp)
                lt0_ev = pev_pool.tile([128, DL], MMD, tag="pev",
                                       name="pev_lt0")

                def proj_unit(dc):
                    ps_p = ps_misc_p.tile([128, 512], F32, tag="m",
                                          name="ps_pi")
                    for ke in range(4):
                        nc.tensor.matmul(
                            ps_p[:],
                            o_sb[ke][:, 0:128],
                            wo_sb[:, ke * DL + dc * 512:
                                  ke * DL + (dc + 1) * 512],
                            start=(ke == 0), stop=(ke == 3))
                    nc.vector.tensor_copy(
                        lt0_ev[:, dc * 512:(dc + 1) * 512], ps_p[:])

                a_ts = {}
                pending = None   # (lc, h, acc, ps_os, av_pair) of prev head
                for lc in range(NLC):
                    a_ts[lc] = [a_pool.tile([128, 1024], MMD, tag=f"a{g}",
                                            name=f"a{g}")
                                for g in range(NSTP)]
                    if lc + 1 < NLC:
                        load_xq(lc + 1)
                        make_qt(lc + 1)
                    # spread the 4MB Wo prefetch across the early l-chunks
                    if 1 <= lc <= 4:
                        ke = lc - 1
                        nc.sync.dma_start(
                            wo_sb[:, ke * DL:(ke + 1) * DL],
                            wot[ke * 128:(ke + 1) * 128, :])
                    for h in range(2):
                        ins = {}
                        if pending is not None:
                            plc, ph_, pacc, pos, pav = pending
                            for i, stp in enumerate(
                                    range(NSTP - AV_DELAY, NSTP)):
                                ins.setdefault(i, []).append(
                                    lambda pav=pav, s=stp: pav(s))
                            ins.setdefault(AV_DELAY, []).append(
                                lambda a=(plc, ph_, pacc, pos):
                                attn_fin(*a))
                        if lc + 1 < NLC:
                            # with no pending tail (first head), the qproj
                            # chains are the only ramp filler -- use the
                            # early slots
                            s0, s1 = ((1, 3) if pending is None else
                                      (AV_DELAY + 1, AV_DELAY + 2))
                            ins.setdefault(s0, []).append(
                                lambda l=lc + 1, hh=h: qproj_chain(
                                    l, hh, 0, qt_tiles[l][hh]))
                            ins.setdefault(s1, []).append(
                                lambda l=lc + 1, hh=h: qproj_chain(
                                    l, hh, 1, qt_tiles[l][hh]))
                        else:
                            # last l-chunk: first out-proj band's units as
                            # ramp filler instead of qproj
                            ins.setdefault(0, []).append(
                                lambda dc=2 * h: proj_unit(dc))
                            ins.setdefault(2, []).append(
                                lambda dc=2 * h + 1: proj_unit(dc))
                        acc, ps_os, av = attn_head_main(
                            lc, h, qt_tiles[lc][h], a_ts[lc], ins)
                        pending = (lc, h, acc, ps_os, av)
                # last head: emit its tail + finalize directly
                plc, ph_, pacc, pos, pav = pending
                for stp in range(NSTP - AV_DELAY, NSTP):
                    pav(stp)
                attn_fin(plc, ph_, pacc, pos)
                if DEBUG_DUMP:
                    for m in range(4):
                        nc.sync.dma_start(
                            o_dump[m * 128:(m + 1) * 128, :], o_sb[m][:])

            # ---- out-projection: partial = outT.T @ WoT -> DRAM ----
            with ExitStack() as ph:
                ph.enter_context(nc.named_scope("proj"))
                psp = ph.enter_context(
                    tc.tile_pool(name="psp", bufs=4, space="PSUM"))
                for lt in range(NLT):
                    if lt == 0:
                        ev = lt0_ev     # dc 0-3 already done in attn ramps
                        dcs = range(4, NDC)
                    else:
                        ev = pev_pool.tile([128, DL], MMD, tag="pev",
                                           name="pev")
                        dcs = range(NDC)
                    for dc in dcs:
                        ps_p = psp.tile([128, 512], F32, tag="ps_p",
                                        name="ps_p")
                        for ke in range(4):
                            nc.tensor.matmul(
                                ps_p[:],
                                o_sb[ke][:, lt * 128:(lt + 1) * 128],
                                wo_sb[:, ke * DL + dc * 512:
                                      ke * DL + (dc + 1) * 512],
                                start=(ke == 0), stop=(ke == 3))
                        if dc % 2 == 0:
                            nc.vector.tensor_copy(
                                ev[:, dc * 512:(dc + 1) * 512], ps_p[:])
                        else:
                            nc.scalar.activation(
                                ev[:, dc * 512:(dc + 1) * 512], ps_p[:],
                                AF.Copy)
                        if lt == NLT - 1:
                            nc.sync.dma_start(
                                out_d[lt * 128:(lt + 1) * 128,
                                      dc * 512:(dc + 1) * 512],
                                ev[:, dc * 512:(dc + 1) * 512])
                    if lt < NLT - 1:
                        nc.sync.dma_start(out_d[lt * 128:(lt + 1) * 128, :],
                                          ev[:])

    nc.compile()
    return nc


def _get_nc():
    if "nc" not in _CACHE:
        _CACHE["nc"] = _build_nc()
    return _CACHE["nc"]


def _build_in_maps(inputs):
    return _prep(**{k: inputs[k] for k in (
        "target_embedding", "source_embedding", "value_embedding",
        "Wq", "bq", "Wk", "bk", "Wv", "bv", "Wo")})


def _prep(target_embedding, source_embedding, value_embedding,
          Wq, bq, Wk, bk, Wv, bv, Wo):
    import ml_dtypes
    mmd = ml_dtypes.bfloat16 if MM_DTYPE == "bf16" else np.float32
    f8 = ml_dtypes.float8_e4m3
    f32 = np.float32

    def to8(a, s):
        return np.clip(a * s, -240.0, 240.0).astype(f8)

    def sbuf_chunks(a, nk, w):
        # a [nk*128, nch*w] -> [nch*128, nk*w]:
        # out[c*128+p, k*w+x] = a[k*128+p, c*w+x]
        nkp, total = a.shape
        nch = total // w
        return np.ascontiguousarray(
            a.reshape(nk, 128, nch, w).transpose(2, 1, 0, 3).reshape(
                nch * 128, nk * w))

    def weight_rows(a, nk):
        # a [nk*128, e] -> [128, nk*e]: out[p, k*e+x] = a[k*128+p, x]
        e = a.shape[1]
        return np.ascontiguousarray(
            a.reshape(nk, 128, e).transpose(1, 0, 2).reshape(128, nk * e))

    X = np.asarray(target_embedding, f32).reshape(BL, D)
    xt = X.T                                             # [D, BL]
    stf = np.zeros((TP, S), f32)
    stf[:T] = np.asarray(source_embedding, f32).T
    vtf = np.zeros((TP, S), f32)
    vtf[:T] = np.asarray(value_embedding, f32).T
    vtf[T] = 1.0                                         # v-bias ones row
    WqT = np.asarray(Wq, f32).T                          # [D, H*E]
    WkT = np.asarray(Wk, f32).T                          # [T, H*E]
    WvT = np.asarray(Wv, f32).T                          # [T, H*E]
    WoT = np.asarray(Wo, f32).T                          # [H*E, DL]
    bq = np.asarray(bq, f32)
    bk = np.asarray(bk, f32)
    bv = np.asarray(bv, f32)

    xt_c = sbuf_chunks(to8(xt, S_X), NKD, 512)           # [NLC*128, NKD*512]
    st_c = sbuf_chunks(to8(stf, S_SE), NKT, 512)         # [NSC*128, NKT*512]
    vt_c = sbuf_chunks(vtf.astype(mmd), NKT, 512)
    in_maps = []
    for i in range(NCORES):
        sl = slice(i * EC, (i + 1) * EC)
        wkt_i = np.zeros((TP, EC), f32)
        wkt_i[:T] = WkT[:, sl]
        wvt_i = np.zeros((TP, EC), f32)
        wvt_i[:T] = WvT[:, sl]
        wvt_i[T] = bv[sl]
        bqbk = np.zeros((128, 8), f32)
        bqbk[:, 0:4] = (bq[sl] * S_QO).reshape(4, 128).T
        bqbk[:, 4:8] = (bk[sl] * S_KO).reshape(4, 128).T
        in_maps.append({
            "xt": xt_c,
            "st": st_c,
            "vt": vt_c,
            "wqt": weight_rows(to8(np.ascontiguousarray(WqT[:, sl]), S_WQ),
                               NKD),
            "wkt": weight_rows(to8(wkt_i, S_WK), NKT),
            "wvt": weight_rows(wvt_i.astype(mmd), NKT),
            "wot": np.ascontiguousarray(WoT[sl, :]).astype(mmd),
            "bqbk": bqbk,
        })
    return in_maps


def kernel(target_embedding, source_embedding, value_embedding,
           Wq, bq, Wk, bk, Wv, bv, Wo, bo):
    from concourse.bass_utils import run_bass_kernel_spmd

    in_maps = _prep(target_embedding, source_embedding, value_embedding,
                    Wq, bq, Wk, bk, Wv, bv, Wo)
    _CACHE["in_maps"] = in_maps
    nc = _get_nc()
    res = run_bass_kernel_spmd(nc, in_maps, list(range(NCORES)))

    acc = np.zeros((BL, DL), np.float32)
    for i in range(NCORES):
        acc += np.asarray(res.results[i]["out"]).astype(np.float32)
    out = (acc + np.asarray(bo, np.float32)[None, :]).astype(np.float32)
    return out.reshape(B, L, DL)


# revision 34
# speedup vs baseline: 1.1190x; 1.1190x over previous
"""Trainium2 Bass kernel for nn_MHAttentionLayer_64587718197528.

Reference computation (B=4, L=1024, D_MODEL=1024, S=2048, T=NUM_TOKENS=1000,
H=16, E=256, D_LLM=4096):
    q = (X @ Wq.T + bq)            [B*L, H*E]      X = target_embedding
    k = (SE @ Wk.T + bk)           [S, H*E]        SE = source_embedding
    v = (VE @ Wv.T + bv)           [S, H*E]        VE = value_embedding
    scores[b,h,l,s] = q . k / 16 ; A = softmax_s ; out = A @ v
    y = out @ Wo.T + bo            [B*L, D_LLM]

Sharding: tensor-parallel over heads. Core i owns heads {2i, 2i+1} (an
e-slice of 512 of the H*E dim). Each core computes its q/k/v projections,
attention for its 2 heads, and a partial out-projection
  partial_i = attn_out_i @ Wo[:, sl_i].T          [B*L, D_LLM]
The host sums the 8 partials and adds bo (linearity of the projection).

Precision strategy (gate is rel 2e-2; measured 1.85e-2):
  fp8e4 + DoubleRow (2x PE rate, HW-verified 2.03x) for the q-projection,
  k-projection and the scores matmul -- their quantization error is
  attenuated by the softmax. The v-projection, AV and out-projection
  matmuls stay bf16 (fp8 there costs 2-4e-2 alone). fp8 operands carry
  scales chosen so maxima sit at ~150-170 of the e4m3 max 240 (TRN e4m3
  overflows to Inf at >240); descales fold into the eviction ACTs and
  the exp scale.

All inputs are pre-rearranged HOST-SIDE into the exact SBUF tile layouts
(chunk-major, partition rows of 4-8KB) so every DMA is a plain 2D copy
with maximal packet sizes -- the natural gather layouts ran the DMA
engines at ~50% utilization and made the KV phase DMA-bound. Streams are
spread over three queues: st/wq/xq0 on the Scalar engine's queue, vt on
the GpSimd engine's, everything else on Sync.

Phases:
  KV:   all four kproj chain groups first (fp8 DR, they need only the
        2MB st stream), then the v-chains (bf16) at the rate the vt
        stream arrives. kT lives per head as [128, 2*S] fp8 (free layout
        (et, s)); v as 4 tiles [128, 4*EC] bf16. Wo is NOT prefetched
        here (it competed with st/vt); it loads during attn.
  Attn: flat pipeline over the 16 (l-chunk, head) units. Per head:
        8 score steps, each 2 fp8-DR matmuls (kt stationary [128,2,128],
        qt moving [128,2,512]) into a double-buffered PSUM pair, exp on
        ACT (scale folds the fp8 descales; no max subtraction), AV bf16
        software-pipelined AV_DELAY steps behind, denominators via DVE
        accumulation + ones-matmul partition-reduce-broadcast +
        reciprocal. The exp chain lags the DR scores during each head's
        ramp (first ~3 steps), so the PREVIOUS head's AV tail (3 av
        pairs), its softmax finalize, and the next l-chunk's qproj
        chains are emitted inside the ramp as filler.
  Proj: partial = outT.T @ WoT per [128,512] tile (bf16), lt-outer so
        each 128-row band leaves as one 1MB DMA, PSUM evictions
        alternating between Scalar and Vector engines.
"""
import numpy as np

# ---- problem constants (hardcoded per contract) ----
B, L, D = 4, 1024, 1024
S, T = 2048, 1000
H, E = 16, 256
DL = 4096
BL = B * L            # 4096 query rows
EC = 512              # e-slice per core (2 heads)
NCORES = 8
TP = 1024             # T zero-padded (includes the v-bias ones row at 1000)
NKD = 8               # k-tiles for D=1024
NKT = 8               # k-tiles for TP=1024
NLC = BL // 512       # 8 l-chunks
NSC = S // 512        # 4 s-chunks (KV phase)

_CACHE = {}
MM_DTYPE = "bf16"     # dtype of the non-fp8 matmul pipeline
AV_DELAY = 3          # AV matmuls trail the scores by this many steps
DEBUG_DUMP = False    # extra DRAM outputs (qt/kt/o) for numerics debugging

# fp8 scales (host-side pre-scaling; descales fold into ACT scale args)
S_X = 32.0            # target_embedding (|X| < 5.3 -> < 170)
S_WQ = 4096.0         # Wq (|Wq| <= 1/32 -> <= 128)
S_SE = 32.0           # source_embedding
S_WK = 4096.0         # Wk (|Wk| <= 1/sqrt(1000) -> <= 130)
S_QO = 48.0           # q in fp8 (|q| < 3.3 -> < 160)
S_KO = 48.0           # k in fp8
SC_EVQ = S_QO / (S_X * S_WQ)
SC_EVK = S_KO / (S_SE * S_WK)
EXP_SCALE = 0.0625 / (S_QO * S_KO)


def _build_nc():
    from contextlib import ExitStack

    import concourse.tile as tile
    from concourse import bacc, bass_isa, mybir

    F32 = mybir.dt.float32
    F32R = mybir.dt.float32r
    MMD = mybir.dt.bfloat16 if MM_DTYPE == "bf16" else F32R
    FP8 = mybir.dt.float8e4
    AF = mybir.ActivationFunctionType
    MUL = mybir.AluOpType.mult
    ADD = mybir.AluOpType.add
    DR = mybir.MatmulPerfMode.DoubleRow

    nc = bacc.Bacc("TRN2", target_bir_lowering=False, debug=False,
                   num_devices=NCORES)

    # all inputs pre-rearranged host-side to SBUF layouts (see _prep)
    xt = nc.dram_tensor("xt", [NLC * 128, NKD * 512], FP8,
                        kind="ExternalInput")
    st = nc.dram_tensor("st", [NSC * 128, NKT * 512], FP8,
                        kind="ExternalInput")
    vt = nc.dram_tensor("vt", [NSC * 128, NKT * 512], MMD,
                        kind="ExternalInput")
    wqt = nc.dram_tensor("wqt", [128, NKD * EC], FP8, kind="ExternalInput")
    wkt = nc.dram_tensor("wkt", [128, NKT * EC], FP8, kind="ExternalInput")
    wvt = nc.dram_tensor("wvt", [128, NKT * EC], MMD, kind="ExternalInput")
    wot = nc.dram_tensor("wot", [EC, DL], MMD, kind="ExternalInput")
    bqbk_d = nc.dram_tensor("bqbk", [128, 8], F32, kind="ExternalInput")
    out_d = nc.dram_tensor("out", [BL, DL], MMD, kind="ExternalOutput")
    if DEBUG_DUMP:
        kt_dump = nc.dram_tensor("kt_dump", [256, 2 * S], FP8,
                                 kind="ExternalOutput")
        qt_dump = nc.dram_tensor("qt_dump", [256, 1024], FP8,
                                 kind="ExternalOutput")
        o_dump = nc.dram_tensor("o_dump", [EC, BL], MMD,
                                kind="ExternalOutput")

    NLT = BL // 128    # 32 l-tiles
    NST = S // 128     # 16 s-tiles
    NDC = DL // 512    # 8 out-proj chunks
    NSTP = NST // 2    # 8 score steps (2 s-tiles each)

    with tile.TileContext(nc) as tc:
        with ExitStack() as root:
            root.enter_context(
                nc.allow_low_precision(reason="bf16/fp8 matmul pipeline"))

            # ---- persistent pools ----
            consts = root.enter_context(tc.tile_pool(name="consts", bufs=1))
            kvp = root.enter_context(tc.tile_pool(name="kv", bufs=1))
            outp = root.enter_context(tc.tile_pool(name="outT", bufs=1))

            ones_f = consts.tile([128, 128], F32, name="ones_f")
            nc.vector.memset(ones_f[:], 1.0)
            ones_m = consts.tile([128, 128], MMD, name="ones_m")
            nc.vector.tensor_copy(ones_m[:], ones_f[:])
            ones_r = consts.tile([128, 128], F32R, name="ones_r")
            nc.vector.tensor_copy(ones_r[:], ones_f[:])

            bqbk_t = consts.tile([128, 8], F32, name="bqbk_t")
            bq_t = bqbk_t[:, 0:4]
            bk_t = bqbk_t[:, 4:8]

            # kT: per head [128, 2*S] fp8 (free layout (et, s));
            # v: 4 tiles [128, 4*EC] bf16 (4 s-tiles each)
            kt_sb = [kvp.tile([128, 2 * S], FP8, name=f"kt{h}", tag=f"kt{h}")
                     for h in range(2)]
            v_sb = [kvp.tile([128, 4 * EC], MMD, name=f"v{g}", tag=f"v{g}")
                    for g in range(4)]
            # outT: 4 e-tiles x [128, BL]
            o_sb = [outp.tile([128, BL], MMD, name=f"oT{m}", tag=f"oT{m}")
                    for m in range(4)]

            wq_pool = root.enter_context(tc.tile_pool(name="wq", bufs=1))
            wo_pool = root.enter_context(tc.tile_pool(name="wo", bufs=1))
            xq_pool = root.enter_context(tc.tile_pool(name="xq", bufs=2))
            pev_pool = root.enter_context(tc.tile_pool(name="pev", bufs=4))
            # qt + the misc PSUM pool live at root so the first qproj can
            # be emitted inside the KV phase (between k- and v-chains,
            # while the vt stream is still arriving)
            qt_pool = root.enter_context(tc.tile_pool(name="qtp", bufs=2))
            ps_misc_p = root.enter_context(
                tc.tile_pool(name="ps_misc", bufs=2, space="PSUM"))
            wq_t = []
            wo_sb = wo_pool.tile([128, 4 * DL], MMD, name="wo_sb")

            qt_tiles = {}

            def make_qt(lc):
                qt_tiles[lc] = [
                    qt_pool.tile([128, 2 * 512], FP8, tag=f"qt{h}",
                                 name=f"qt{h}_{lc}") for h in range(2)]

            # PE clock warm-up: the tensor engine ramps 0.65 -> 2.4GHz
            # over ~3-4us of sustained work; burn that in during the
            # initial DMA wait (~11us) on dummy matmuls nothing reads.
            # Uses the root misc PSUM pool (no extra bank).
            for wchain in range(6):
                wt = ps_misc_p.tile([128, 512], F32, tag="m", name="warm")
                for r in range(8):
                    nc.tensor.matmul(wt[:, 0:128], ones_m[:], ones_m[:],
                                     start=(r == 0), stop=(r == 7))

            def qproj_chain(lc, half, mh, qt_t):
                # one fp8 DoubleRow chain (one e-tile of the head)
                xq_t = xq_tiles[lc]
                wqv = wq_t[0][:].rearrange("p (k e) -> p k e", k=NKD)
                xqv = xq_t[:].rearrange("p (k c) -> p k c", k=NKD)
                m = half * 2 + mh
                ps_q = ps_misc_p.tile([128, 512], F32, tag="m",
                                      name=f"psq{mh}")
                for kj in range(NKD // 2):
                    nc.tensor.matmul(
                        ps_q[:],
                        wqv[:, 2 * kj:2 * kj + 2,
                            m * 128:(m + 1) * 128],
                        xqv[:, 2 * kj:2 * kj + 2, :],
                        start=(kj == 0), stop=(kj == NKD // 2 - 1),
                        perf_mode=DR)
                nc.scalar.activation(
                    qt_t[:, mh * 512:(mh + 1) * 512], ps_q[:],
                    AF.Identity, bias=bq_t[:, m:m + 1], scale=SC_EVQ)

            xq_tiles = {}

            def load_xq(lc, eng=None):
                t = xq_pool.tile([128, NKD * 512], FP8, tag="xq",
                                 name=f"xq{lc}")
                (eng or nc.sync).dma_start(
                    t[:], xt.ap()[lc * 128:(lc + 1) * 128, :])
                xq_tiles[lc] = t

            # ---- phase KV ----
            with ExitStack() as ph:
                ph.enter_context(nc.named_scope("kvproj"))
                wkv_pool = ph.enter_context(tc.tile_pool(name="wkv", bufs=1))
                sk_pool = ph.enter_context(tc.tile_pool(name="sk", bufs=4))
                sv_pool = ph.enter_context(tc.tile_pool(name="sv", bufs=4))
                psk = ph.enter_context(
                    tc.tile_pool(name="psk", bufs=1, space="PSUM"))
                psv = ph.enter_context(
                    tc.tile_pool(name="psv", bufs=1, space="PSUM"))
                wk_sb = wkv_pool.tile([128, NKT * EC], FP8, name="wk_sb")
                wv_sb = wkv_pool.tile([128, NKT * EC], MMD, name="wv_sb")

                def load_st(sc, eng=None):
                    t = sk_pool.tile([128, NKT * 512], FP8, tag="stg",
                                     name=f"stg{sc}")
                    (eng or nc.sync).dma_start(
                        t[:], st.ap()[sc * 128:(sc + 1) * 128, :])
                    return t

                def load_vt(sc, eng=None):
                    t = sv_pool.tile([128, NKT * 512], MMD, tag="vtg",
                                     name=f"vtg{sc}")
                    (eng or nc.sync).dma_start(
                        t[:], vt.ap()[sc * 128:(sc + 1) * 128, :])
                    return t

                # Per-core DMA bandwidth (~360GB/s) is shared across the
                # queues, so per-queue priority = issue order and the
                # three queues (Sync/Scalar/GpSimd) run in parallel.
                # Deadline order: wk+st (k-chains) < wq/xq0 (qproj0) <
                # wv+vt0 < vt1..3 (v-chains). Every DMA is a whole-chunk
                # 2D copy (4-8KB descriptors -- small descriptors lose
                # queue arbitration). k-chains run in st arrival order
                # 0,2,1,3 (two st streams in parallel).
                st_ts = {0: load_st(0, eng=nc.gpsimd),
                         2: load_st(2, eng=nc.scalar),
                         1: load_st(1, eng=nc.gpsimd),
                         3: load_st(3, eng=nc.scalar)}
                nc.sync.dma_start(wk_sb[:], wkt.ap())
                nc.sync.dma_start(bqbk_t[:], bqbk_d.ap())
                nc.sync.dma_start(wv_sb[:], wvt.ap())
                wq_sb = wq_pool.tile([128, NKD * EC], FP8, name="wq_sb")
                nc.scalar.dma_start(wq_sb[:], wqt.ap())
                wq_t.append(wq_sb)
                load_xq(0, eng=nc.scalar)
                vt_ts = {0: load_vt(0, eng=nc.gpsimd),
                         1: load_vt(1, eng=nc.sync),
                         2: load_vt(2, eng=nc.gpsimd),
                         3: load_vt(3, eng=nc.scalar)}
                # all k-chains first (m-major so evictions start early)
                wkv = wk_sb[:].rearrange("p (k e) -> p k e", k=NKT)
                for sc in (0, 2, 1, 3):
                    ps_k = [psk.tile([128, 512], F32, tag=f"psk{m}",
                                     name=f"psk{m}") for m in range(4)]
                    stv = st_ts[sc][:].rearrange("p (k c) -> p k c", k=NKT)
                    for m in range(4):
                        for kj in range(NKT // 2):
                            nc.tensor.matmul(
                                ps_k[m][:],
                                wkv[:, 2 * kj:2 * kj + 2,
                                    m * 128:(m + 1) * 128],
                                stv[:, 2 * kj:2 * kj + 2, :],
                                start=(kj == 0), stop=(kj == NKT // 2 - 1),
                                perf_mode=DR)
                        nc.scalar.activation(
                            kt_sb[m // 2][:, (m % 2) * S + sc * 512:
                                          (m % 2) * S + (sc + 1) * 512],
                            ps_k[m][:], AF.Identity, bias=bk_t[:, m:m + 1],
                            scale=SC_EVK)
                # l-chunk 0's q-projection here: PE work that needs only
                # wq/xq0, bridging the window where vt is still streaming
                make_qt(0)
                for half in range(2):
                    for mh in range(2):
                        qproj_chain(0, half, mh, qt_tiles[0][half])
                # v-chains at the rate the vt stream arrives (2 PSUM
                # tiles: j and j+2 share a bank, eviction-paced)
                for sc in range(NSC):
                    vt_t = vt_ts[sc]
                    for j in range(4):
                        pv = psv.tile([128, 512], F32, tag=f"psv{j % 2}",
                                      name=f"psv{j % 2}")
                        for kk in range(NKT):
                            nc.tensor.matmul(
                                pv[:],
                                vt_t[:, kk * 512 + j * 128:
                                     kk * 512 + (j + 1) * 128],
                                wv_sb[:, kk * EC:(kk + 1) * EC],
                                start=(kk == 0), stop=(kk == NKT - 1))
                        nc.scalar.activation(
                            v_sb[sc][:, j * EC:(j + 1) * EC], pv[:],
                            AF.Copy)
                if DEBUG_DUMP:
                    for h in range(2):
                        nc.sync.dma_start(
                            kt_dump[h * 128:(h + 1) * 128, :], kt_sb[h][:])

            # ---- fused attention phase ----
            with ExitStack() as ph:
                ph.enter_context(nc.named_scope("attn"))
                a_pool = ph.enter_context(tc.tile_pool(name="ap", bufs=1))
                acc_pool = ph.enter_context(tc.tile_pool(name="accp", bufs=2))
                bc_pool = ph.enter_context(tc.tile_pool(name="bcp", bufs=2))
                # PSUM budget (8 banks): misc (qproj + denom, root) 2,
                # scores double-buffered 4, attn-out accumulators 2.
                ps_sT_p = ph.enter_context(
                    tc.tile_pool(name="ps_sT", bufs=2, space="PSUM"))
                ps_o_p = ph.enter_context(
                    tc.tile_pool(name="ps_o", bufs=2, space="PSUM"))

                def attn_head_main(lc, h, qt_t, a_t, inserts=None):
                    # scoresT via one fp8 DoubleRow matmul per s-tile;
                    # one exp per 1024 columns; AV (bf16) pipelined
                    # AV_DELAY steps behind. The AV tail (last AV_DELAY
                    # steps) is NOT emitted here -- the caller threads it
                    # into the next head's ramp as PE filler.
                    acc = acc_pool.tile([128, 1024], F32, tag="acc",
                                        name="acc")
                    ps_os = [ps_o_p.tile([128, 512], F32, tag="ps_o",
                                         name="ps_o") for _ in range(2)]
                    ktv = kt_sb[h][:].rearrange("p (et s) -> p et s", et=2)
                    qtv = qt_t[:].rearrange("p (et l) -> p et l", et=2)

                    def av_pair(stp):
                        for et in range(2):
                            for sub in range(2):
                                stt = 2 * stp + sub
                                nc.tensor.matmul(
                                    ps_os[et][:],
                                    v_sb[stt // 4][:, (stt % 4) * EC + h * E
                                                   + et * 128:
                                                   (stt % 4) * EC + h * E
                                                   + (et + 1) * 128],
                                    a_t[stp][:, sub * 512:(sub + 1) * 512],
                                    start=(stt == 0), stop=(stt == NST - 1))

                    for stp in range(NSTP):
                        ps_sT = ps_sT_p.tile([128, 1024], F32, tag="ps_sT",
                                             name="ps_sT")
                        for sub in range(2):
                            stt = 2 * stp + sub
                            nc.tensor.matmul(
                                ps_sT[:, sub * 512:(sub + 1) * 512],
                                ktv[:, :, stt * 128:(stt + 1) * 128],
                                qtv,
                                start=True, stop=True, perf_mode=DR)
                        a_ap = a_t[stp][:]
                        nc.scalar.activation(a_ap, ps_sT[:], AF.Exp,
                                             scale=EXP_SCALE)
                        # accumulate denominator on DVE
                        if stp == 0:
                            nc.vector.tensor_copy(acc[:], a_ap)
                        else:
                            nc.vector.tensor_tensor(acc[:], acc[:], a_ap,
                                                    ADD)
                        if inserts and stp in inserts:
                            for fn in inserts[stp]:
                                fn()
                        if stp >= AV_DELAY:
                            av_pair(stp - AV_DELAY)
                    return acc, ps_os, av_pair

                def attn_fin(lc, h, acc, ps_os):
                    # softmax denominators: fold acc halves on DVE into an
                    # f32r tile (fp22-read: matmul stays 1 cyc/row at
                    # moving 512, ~64x less rounding than the old bf16
                    # fold), one ones-matmul = partition reduction AND
                    # broadcast, reciprocal on DVE.
                    accb = bc_pool.tile([128, 512], F32R, tag="accb",
                                        name="accb")
                    nc.vector.tensor_tensor(accb[:], acc[:, 0:512],
                                            acc[:, 512:1024], ADD)
                    ps_b = ps_misc_p.tile([128, 512], F32, tag="m",
                                          name="ps_b")
                    nc.tensor.matmul(ps_b[:], ones_r[:], accb[:],
                                     start=True, stop=True)
                    bc = bc_pool.tile([128, 512], F32, tag="bc", name="bc")
                    nc.vector.reciprocal_approx_fast(out=bc[:], in_=ps_b[:])
                    for et in range(2):
                        m = 2 * h + et
                        nc.vector.tensor_tensor(
                            o_sb[m][:, lc * 512:(lc + 1) * 512],
                            ps_os[et][:], bc[:], MUL)

                # Flat pipeline over the 16 heads. Each head's ramp hosts
                # the PREVIOUS head's AV tail (steps 0..AV_DELAY-1), its
                # finalize (step AV_DELAY, always before this head's own
                # first av_pair), and the next l-chunk's qproj chains.
                # (qproj for l-chunk 0 already ran inside the KV phase.)
                if DEBUG_DUMP:
                    for h in range(2):
                        nc.sync.dma_start(
                            qt_dump[h * 128:(h + 1) * 128, :],
                            qt_tiles[0][h][:])

                # out-projection units for the first l-band, usable as PE
                # filler in the last l-chunk's ramps (no qproj there);
                # evictions on DVE (ACT is exp-critical in the ramp)
                lt0_ev = pev_pool.tile([128, DL], MMD, tag="pev",
                                       name="pev_lt0")

                def proj_unit(dc):
                    ps_p = ps_misc_p.tile([128, 512], F32, tag="m",
                                          name="ps_pi")
                    for ke in range(4):
                        nc.tensor.matmul(
                            ps_p[:],
                            o_sb[ke][:, 0:128],
                            wo_sb[:, ke * DL + dc * 512:
                                  ke * DL + (dc + 1) * 512],
                            start=(ke == 0), stop=(ke == 3))
                    nc.vector.tensor_copy(
                        lt0_ev[:, dc * 512:(dc + 1) * 512], ps_p[:])

                a_ts = {}
                pending = None   # (lc, h, acc, ps_os, av_pair) of prev head
                for lc in range(NLC):
                    a_ts[lc] = [a_pool.tile([128, 1024], MMD, tag=f"a{g}",
                                            name=f"a{g}")
                                for g in range(NSTP)]
                    if lc + 1 < NLC:
                        load_xq(lc + 1)
                        make_qt(lc + 1)
                    # spread the 4MB Wo prefetch across the early l-chunks
                    if 1 <= lc <= 4:
                        ke = lc - 1
                        nc.sync.dma_start(
                            wo_sb[:, ke * DL:(ke + 1) * DL],
                            wot[ke * 128:(ke + 1) * 128, :])
                    for h in range(2):
                        ins = {}
                        if pending is not None:
                            plc, ph_, pacc, pos, pav = pending
                            for i, stp in enumerate(
                                    range(NSTP - AV_DELAY, NSTP)):
                                ins.setdefault(i, []).append(
                                    lambda pav=pav, s=stp: pav(s))
                            ins.setdefault(AV_DELAY, []).append(
                                lambda a=(plc, ph_, pacc, pos):
                                attn_fin(*a))
                        if lc + 1 < NLC:
                            # with no pending tail (first head), the qproj
                            # chains are the only ramp filler -- use the
                            # early slots
                            s0, s1 = ((1, 3) if pending is None else
                                      (AV_DELAY + 1, AV_DELAY + 2))
                            ins.setdefault(s0, []).append(
                                lambda l=lc + 1, hh=h: qproj_chain(
                                    l, hh, 0, qt_tiles[l][hh]))
                            ins.setdefault(s1, []).append(
                                lambda l=lc + 1, hh=h: qproj_chain(
                                    l, hh, 1, qt_tiles[l][hh]))
                        else:
                            # last l-chunk: first out-proj band's units as
                            # ramp filler instead of qproj
                            ins.setdefault(0, []).append(
                                lambda dc=2 * h: proj_unit(dc))
                            ins.setdefault(2, []).append(
                                lambda dc=2 * h + 1: proj_unit(dc))
                        acc, ps_os, av = attn_head_main(
                            lc, h, qt_tiles[lc][h], a_ts[lc], ins)
                        pending = (lc, h, acc, ps_os, av)
                # last head: emit its tail + finalize directly
                plc, ph_, pacc, pos, pav = pending
                for stp in range(NSTP - AV_DELAY, NSTP):
                    pav(stp)
                attn_fin(plc, ph_, pacc, pos)
                if DEBUG_DUMP:
                    for m in range(4):
                        nc.sync.dma_start(
                            o_dump[m * 128:(m + 1) * 128, :], o_sb[m][:])

            # ---- out-projection: partial = outT.T @ WoT -> DRAM ----
            with ExitStack() as ph:
                ph.enter_context(nc.named_scope("proj"))
                psp = ph.enter_context(
                    tc.tile_pool(name="psp", bufs=4, space="PSUM"))
                for lt in range(NLT):
                    if lt == 0:
                        ev = lt0_ev     # dc 0-3 already done in attn ramps
                        dcs = range(4, NDC)
                    else:
                        ev = pev_pool.tile([128, DL], MMD, tag="pev",
                                           name="pev")
                        dcs = range(NDC)
                    for dc in dcs:
                        ps_p = psp.tile([128, 512], F32, tag="ps_p",
                                        name="ps_p")
                        for ke in range(4):
                            nc.tensor.matmul(
                                ps_p[:],
                                o_sb[ke][:, lt * 128:(lt + 1) * 128],
                                wo_sb[:, ke * DL + dc * 512:
                                      ke * DL + (dc + 1) * 512],
                                start=(ke == 0), stop=(ke == 3))
                        if dc % 2 == 0:
                            nc.vector.tensor_copy(
                                ev[:, dc * 512:(dc + 1) * 512], ps_p[:])
                        else:
                            nc.scalar.activation(
                                ev[:, dc * 512:(dc + 1) * 512], ps_p[:],
                                AF.Copy)
                        if lt == NLT - 1:
                            nc.sync.dma_start(
                                out_d[lt * 128:(lt + 1) * 128,
                                      dc * 512:(dc + 1) * 512],
                                ev[:, dc * 512:(dc + 1) * 512])
                    if lt < NLT - 1:
                        nc.sync.dma_start(out_d[lt * 128:(lt + 1) * 128, :],
                                          ev[:])

    nc.compile()
    return nc


def _get_nc():
    if "nc" not in _CACHE:
        _CACHE["nc"] = _build_nc()
    return _CACHE["nc"]


def _build_in_maps(inputs):
    return _prep(**{k: inputs[k] for k in (
        "target_embedding", "source_embedding", "value_embedding",
        "Wq", "bq", "Wk", "bk", "Wv", "bv", "Wo")})


def _prep(target_embedding, source_embedding, value_embedding,
          Wq, bq, Wk, bk, Wv, bv, Wo):
    import ml_dtypes
    mmd = ml_dtypes.bfloat16 if MM_DTYPE == "bf16" else np.float32
    f8 = ml_dtypes.float8_e4m3
    f32 = np.float32

    def to8(a, s):
        return np.clip(a * s, -240.0, 240.0).astype(f8)

    def sbuf_chunks(a, nk, w):
        # a [nk*128, nch*w] -> [nch*128, nk*w]:
        # out[c*128+p, k*w+x] = a[k*128+p, c*w+x]
        nkp, total = a.shape
        nch = total // w
        return np.ascontiguousarray(
            a.reshape(nk, 128, nch, w).transpose(2, 1, 0, 3).reshape(
                nch * 128, nk * w))

    def weight_rows(a, nk):
        # a [nk*128, e] -> [128, nk*e]: out[p, k*e+x] = a[k*128+p, x]
        e = a.shape[1]
        return np.ascontiguousarray(
            a.reshape(nk, 128, e).transpose(1, 0, 2).reshape(128, nk * e))

    X = np.asarray(target_embedding, f32).reshape(BL, D)
    xt = X.T                                             # [D, BL]
    stf = np.zeros((TP, S), f32)
    stf[:T] = np.asarray(source_embedding, f32).T
    vtf = np.zeros((TP, S), f32)
    vtf[:T] = np.asarray(value_embedding, f32).T
    vtf[T] = 1.0                                         # v-bias ones row
    WqT = np.asarray(Wq, f32).T                          # [D, H*E]
    WkT = np.asarray(Wk, f32).T                          # [T, H*E]
    WvT = np.asarray(Wv, f32).T                          # [T, H*E]
    WoT = np.asarray(Wo, f32).T                          # [H*E, DL]
    bq = np.asarray(bq, f32)
    bk = np.asarray(bk, f32)
    bv = np.asarray(bv, f32)

    xt_c = sbuf_chunks(to8(xt, S_X), NKD, 512)           # [NLC*128, NKD*512]
    st_c = sbuf_chunks(to8(stf, S_SE), NKT, 512)         # [NSC*128, NKT*512]
    vt_c = sbuf_chunks(vtf.astype(mmd), NKT, 512)
    in_maps = []
    for i in range(NCORES):
        sl = slice(i * EC, (i + 1) * EC)
        wkt_i = np.zeros((TP, EC), f32)
        wkt_i[:T] = WkT[:, sl]
        wvt_i = np.zeros((TP, EC), f32)
        wvt_i[:T] = WvT[:, sl]
        wvt_i[T] = bv[sl]
        bqbk = np.zeros((128, 8), f32)
        bqbk[:, 0:4] = (bq[sl] * S_QO).reshape(4, 128).T
        bqbk[:, 4:8] = (bk[sl] * S_KO).reshape(4, 128).T
        in_maps.append({
            "xt": xt_c,
            "st": st_c,
            "vt": vt_c,
            "wqt": weight_rows(to8(np.ascontiguousarray(WqT[:, sl]), S_WQ),
                               NKD),
            "wkt": weight_rows(to8(wkt_i, S_WK), NKT),
            "wvt": weight_rows(wvt_i.astype(mmd), NKT),
            "wot": np.ascontiguousarray(WoT[sl, :]).astype(mmd),
            "bqbk": bqbk,
        })
    return in_maps


def kernel(target_embedding, source_embedding, value_embedding,
           Wq, bq, Wk, bk, Wv, bv, Wo, bo):
    from concourse.bass_utils import run_bass_kernel_spmd

    in_maps = _prep(target_embedding, source_embedding, value_embedding,
                    Wq, bq, Wk, bk, Wv, bv, Wo)
    _CACHE["in_maps"] = in_maps
    nc = _get_nc()
    res = run_bass_kernel_spmd(nc, in_maps, list(range(NCORES)))

    acc = np.zeros((BL, DL), np.float32)
    for i in range(NCORES):
        acc += np.asarray(res.results[i]["out"]).astype(np.float32)
    out = (acc + np.asarray(bo, np.float32)[None, :]).astype(np.float32)
    return out.reshape(B, L, DL)
